# revision 9
# baseline (speedup 1.0000x reference)
"""Trainium2 Bass kernel for the GIN message-passing GNN (8 NeuronCores).

Strategy
--------
Nodes are relabeled (host-side permutation) to balance per-tile edge load and
sharded contiguously across 8 cores (6272 nodes/core = 49 tiles of 128).
Edges are assigned to the core/tile of their DESTINATION node; the GIN
self-loop term becomes an explicit self-edge in the gather list.  Each layer:

  1. `dma_gather` pulls the bf16 source-node features (256B rows) from a
     replicated HBM table (two halves, since gather indices are int16).
  2. One-hot "S" matrices (built on-chip with is_equal against a constant
     iota row) turn the segment-sum into PE matmuls accumulating agg^T in
     PSUM: agg^T[f,m] = sum_e G[e,f] * [row_e == m].
  3. The GIN MLP (2x Linear+ReLU+BN-eval) runs as two 128x128 matmuls; BN
     affine params are folded into the next linear's weights on the host.
     The degree-dependent bias (from folding BN through the aggregation) is
     added with a tiny K=2 matmul against a per-tile {degree, ones} matrix.
  4. A PE transpose produces the node-major tile, which is DMAd into the
     local slice of the next gather table; an AllGather collective rebuilds
     the replicated table for the next layer.
  5. Graph readouts accumulate in PSUM via one-hot batch matmuls; a final
     AllReduce + classifier (fp32) + log_softmax runs on every core.

The tables store the pre-BN relu outputs ("r-basis"); readouts are fixed up
after the AllReduce with the folded scale and a host-computed n_g * beta
constant.
"""

import numpy as np
import ml_dtypes
from contextlib import ExitStack  # noqa: F401

from concourse import bass, bacc, tile, mybir
from concourse.bass_utils import run_bass_kernel_spmd

bf16 = ml_dtypes.bfloat16
DT = mybir.dt

# ---- problem constants (hardcoded per contest contract)
N0, E0, F, L, M, G, C = 50000, 600000, 128, 4, 2, 64, 10
BN_EPS = 1e-5
CORES, NPC, NT, P = 8, 6272, 49, 128
NPAD, HALF = 50176, 25088
CAPH = 1024                  # per-(tile,half) edge capacity (pad w/ idx 0)
SLOTS16 = CAPH // 16         # 64 int16 index columns per gather
NCHH = CAPH // 128           # 8 chunks per half
NCH = 2 * NCHH               # 16 chunks per tile
NTILES = NT * CORES

_CACHE = {}


def _build_program():
    nc = bacc.Bacc(
        "TRN2",
        target_bir_lowering=False,
        debug=False,
        enable_asserts=False,
        num_devices=CORES,
    )

    # ---------------- I/O ----------------
    xtab = nc.dram_tensor("xtab", [NPAD, F], DT.bfloat16, kind="ExternalInput")
    xloc = nc.dram_tensor("xloc", [NPC, F], DT.bfloat16, kind="ExternalInput")
    idx16 = nc.dram_tensor("idx16", [128, NT * 2 * SLOTS16], DT.int16, kind="ExternalInput")
    rowloc = nc.dram_tensor("rowloc", [128, NT * NCH], DT.float32, kind="ExternalInput")
    degones = nc.dram_tensor("degones", [2, NPC], DT.bfloat16, kind="ExternalInput")
    batchloc = nc.dram_tensor("batchloc", [128, NT], DT.float32, kind="ExternalInput")
    wmlp = nc.dram_tensor("wmlp", [2 * L, F, F], DT.bfloat16, kind="ExternalInput")
    biasl = nc.dram_tensor("biasl", [L, 3, F], DT.bfloat16, kind="ExternalInput")
    wc1 = nc.dram_tensor("wc1", [25, F, F], DT.float32, kind="ExternalInput")
    wc2 = nc.dram_tensor("wc2", [5, F, C], DT.float32, kind="ExternalInput")
    sfix = nc.dram_tensor("sfix", [128, 5], DT.float32, kind="ExternalInput")
    zfix = nc.dram_tensor("zfix", [128, 5 * G], DT.float32, kind="ExternalInput")
    jrow = nc.dram_tensor("jrow", [128, 128], DT.bfloat16, kind="ExternalInput")
    ident = nc.dram_tensor("ident", [128, 128], DT.bfloat16, kind="ExternalInput")
    out_dram = nc.dram_tensor("out", [G, C], DT.float32, kind="ExternalOutput")

    # internal DRAM for collectives
    ccin = [nc.dram_tensor(f"ccin{k}", [NPC, F], DT.bfloat16) for k in range(L - 1)]
    ccout = [
        nc.dram_tensor(f"ccout{k}", [NPAD, F], DT.bfloat16, addr_space="Shared")
        for k in range(L - 1)
    ]
    zrin = nc.dram_tensor("zrin", [128, 5 * G], DT.float32)
    zrout = nc.dram_tensor("zrout", [128, 5 * G], DT.float32, addr_space="Shared")

    AOT = mybir.AluOpType
    ACT = mybir.ActivationFunctionType

    with tile.TileContext(nc) as tc:
        with (
            tc.tile_pool(name="const", bufs=1) as cpool,
            tc.tile_pool(name="stage", bufs=6) as stpool,
            tc.tile_pool(name="smat", bufs=3) as spool,
            tc.tile_pool(name="work", bufs=4) as wpool,
            tc.tile_pool(name="psum", bufs=1, space="PSUM") as pspool,
            tc.tile_pool(name="psumr", bufs=1, space="PSUM") as prpool,
        ):
            # ------- resident constants -------
            idx_sb = cpool.tile([128, NT * 2 * SLOTS16], DT.int16)
            nc.sync.dma_start(idx_sb[:], idx16.ap())
            rowloc_sb = cpool.tile([128, NT * NCH], DT.float32)
            nc.sync.dma_start(rowloc_sb[:], rowloc.ap())
            degones_sb = cpool.tile([2, NPC], DT.bfloat16)
            nc.sync.dma_start(degones_sb[:], degones.ap())
            batchloc_sb = cpool.tile([128, NT], DT.float32)
            nc.sync.dma_start(batchloc_sb[:], batchloc.ap())
            jrow_sb = cpool.tile([128, 128], DT.bfloat16)
            nc.sync.dma_start(jrow_sb[:], jrow.ap())
            ident_sb = cpool.tile([128, 128], DT.bfloat16)
            nc.sync.dma_start(ident_sb[:], ident.ap())
            wmlp_sb = cpool.tile([128, 2 * L * F], DT.bfloat16)
            for ki in range(2 * L):
                nc.sync.dma_start(wmlp_sb[:, ki * F:(ki + 1) * F], wmlp.ap()[ki])
            biasl12_sb = cpool.tile([2, L * F], DT.bfloat16)
            biasl3_sb = cpool.tile([1, L * F], DT.bfloat16)
            for k in range(L):
                nc.sync.dma_start(biasl12_sb[:, k * F:(k + 1) * F], biasl.ap()[k][0:2, :])
                nc.sync.dma_start(biasl3_sb[:, k * F:(k + 1) * F], biasl.ap()[k][2:3, :])
            wc1_sb = cpool.tile([128, 25 * F], DT.float32)
            for ij in range(25):
                nc.sync.dma_start(wc1_sb[:, ij * F:(ij + 1) * F], wc1.ap()[ij])
            wc2_sb = cpool.tile([128, 5 * C], DT.float32)
            for j in range(5):
                nc.sync.dma_start(wc2_sb[:, j * C:(j + 1) * C], wc2.ap()[j])
            sfix_sb = cpool.tile([128, 5], DT.float32)
            nc.sync.dma_start(sfix_sb[:], sfix.ap())
            zfix_sb = cpool.tile([128, 5 * G], DT.float32)
            nc.sync.dma_start(zfix_sb[:], zfix.ap())

            # one-hot batch matrices per tile: B[m, g] = (batchloc[m,t]==g)
            ball_sb = cpool.tile([128, NT * G], DT.bfloat16)
            for t in range(NT):
                nc.vector.tensor_scalar(
                    ball_sb[:, t * G:(t + 1) * G],
                    jrow_sb[:, :G],
                    batchloc_sb[:, t:t + 1],
                    None,
                    AOT.is_equal,
                )

            zr_sb = cpool.tile([128, 5 * G], DT.float32)

            # ---------------- GIN layers ----------------
            for k in range(L):
                table = xtab if k == 0 else ccout[k - 1]
                tap = table.ap()
                psR = prpool.tile([128, G], DT.float32, tag="psR")
                psR0 = prpool.tile([128, G], DT.float32, tag="psR0", name="psR0") if k == 0 else None
                for t in range(NT):
                    stg = [
                        stpool.tile([128, NCHH, 128], DT.bfloat16, tag=f"stg{h}",
                                    name=f"stg{h}")
                        for h in range(2)
                    ]
                    for h in range(2):
                        nc.gpsimd.dma_gather(
                            out_ap=stg[h][:],
                            in_ap=tap[h * HALF:(h + 1) * HALF, :],
                            idxs_ap=idx_sb[:, (t * 2 + h) * SLOTS16:(t * 2 + h + 1) * SLOTS16],
                            num_idxs=CAPH,
                            num_idxs_reg=CAPH,
                            elem_size=F,
                            queue_num=0,
                        )
                    smat = spool.tile([128, NCH, 128], DT.bfloat16, tag="smat")
                    for ch in range(NCH):
                        nc.vector.tensor_scalar(
                            smat[:, ch, :],
                            jrow_sb[:],
                            rowloc_sb[:, t * NCH + ch:t * NCH + ch + 1],
                            None,
                            AOT.is_equal,
                        )
                    psA = pspool.tile([128, 128], DT.float32, tag="psA", bufs=2)
                    for ch in range(NCH):
                        nc.tensor.matmul(
                            psA[:],
                            stg[ch // NCHH][:, ch % NCHH, :],
                            smat[:, ch, :],
                            start=(ch == 0),
                            stop=(ch == NCH - 1),
                        )
                    aggr = wpool.tile([128, 128], DT.bfloat16, tag="aggr")
                    nc.scalar.copy(aggr[:], psA[:])
                    # MLP sublayer 1 (+ degree bias)
                    psB = pspool.tile([128, 128], DT.float32, tag="psB")
                    nc.tensor.matmul(
                        psB[:], wmlp_sb[:, (2 * k) * F:(2 * k + 1) * F], aggr[:],
                        start=True, stop=False,
                    )
                    nc.tensor.matmul(
                        psB[:], biasl12_sb[:, k * F:(k + 1) * F],
                        degones_sb[0:2, t * 128:(t + 1) * 128],
                        start=False, stop=True,
                    )
                    r1 = wpool.tile([128, 128], DT.bfloat16, tag="r1")
                    nc.scalar.activation(r1[:], psB[:], ACT.Relu)
                    # MLP sublayer 2 (+ constant bias)
                    psC = pspool.tile([128, 128], DT.float32, tag="psC")
                    nc.tensor.matmul(
                        psC[:], wmlp_sb[:, (2 * k + 1) * F:(2 * k + 2) * F], r1[:],
                        start=True, stop=False,
                    )
                    nc.tensor.matmul(
                        psC[:], biasl3_sb[:, k * F:(k + 1) * F],
                        degones_sb[0:1, t * 128:(t + 1) * 128],
                        start=False, stop=True,
                    )
                    r2 = wpool.tile([128, 128], DT.bfloat16, tag="r2")
                    nc.scalar.activation(r2[:], psC[:], ACT.Relu)
                    # node-major copy (for table store + readout)
                    psT = pspool.tile([128, 128], DT.bfloat16, tag="psT", bufs=2)
                    nc.tensor.matmul(psT[:], r2[:], ident_sb[:], is_transpose=True)
                    r2nm = wpool.tile([128, 128], DT.bfloat16, tag="r2nm")
                    nc.scalar.copy(r2nm[:], psT[:])
                    if k < L - 1:
                        nc.sync.dma_start(ccin[k].ap()[t * 128:(t + 1) * 128, :], r2nm[:])
                    nc.tensor.matmul(
                        psR[:], r2nm[:], ball_sb[:, t * G:(t + 1) * G],
                        start=(t == 0), stop=(t == NT - 1), skip_group_check=True,
                    )
                    if k == 0:
                        xt_t = stpool.tile([128, 128], DT.bfloat16, tag="xt")
                        nc.sync.dma_start(xt_t[:], xloc.ap()[t * 128:(t + 1) * 128, :])
                        nc.tensor.matmul(
                            psR0[:], xt_t[:], ball_sb[:, t * G:(t + 1) * G],
                            start=(t == 0), stop=(t == NT - 1), skip_group_check=True,
                        )
                nc.scalar.copy(zr_sb[:, (k + 1) * G:(k + 2) * G], psR[:])
                if k == 0:
                    nc.scalar.copy(zr_sb[:, 0:G], psR0[:])
                if k < L - 1:
                    nc.gpsimd.collective_compute(
                        "AllGather",
                        AOT.bypass,
                        replica_groups=[list(range(CORES))],
                        ins=[ccin[k].ap().opt()],
                        outs=[ccout[k].ap().opt()],
                    )

            # ---------------- readout AllReduce + fixup ----------------
            nc.sync.dma_start(zrin.ap()[:], zr_sb[:])
            nc.gpsimd.collective_compute(
                "AllReduce",
                AOT.add,
                replica_groups=[list(range(CORES))],
                ins=[zrin.ap().opt()],
                outs=[zrout.ap().opt()],
            )
            zsum = cpool.tile([128, 5 * G], DT.float32)
            nc.sync.dma_start(zsum[:], zrout.ap()[:])
            zfx = cpool.tile([128, 5 * G], DT.float32)
            for kk in range(5):
                nc.vector.tensor_scalar(
                    zfx[:, kk * G:(kk + 1) * G],
                    zsum[:, kk * G:(kk + 1) * G],
                    sfix_sb[:, kk:kk + 1],
                    None,
                    AOT.mult,
                )
            nc.vector.tensor_tensor(zfx[:], zfx[:], zfix_sb[:], AOT.add)

            # ---------------- classifier (fp32) ----------------
            rc1 = []
            for j in range(5):
                psC1 = pspool.tile([128, G], DT.float32, tag="psA", name="psC1", bufs=2)
                for i in range(5):
                    nc.tensor.matmul(
                        psC1[:], wc1_sb[:, (i * 5 + j) * F:(i * 5 + j + 1) * F],
                        zfx[:, i * G:(i + 1) * G],
                        start=(i == 0), stop=(i == 4),
                    )
                r = cpool.tile([128, G], DT.float32, tag=f"rc1_{j}", name=f"rc1_{j}")
                nc.scalar.activation(r[:], psC1[:], ACT.Relu)
                rc1.append(r)
            psC2 = prpool.tile([G, C], DT.float32, tag="psR0", name="psC2")
            for j in range(5):
                nc.tensor.matmul(
                    psC2[:], rc1[j][:], wc2_sb[:, j * C:(j + 1) * C],
                    start=(j == 0), stop=(j == 4),
                )
            z2sb = cpool.tile([G, C], DT.float32)
            nc.scalar.copy(z2sb[:], psC2[:])
            mx = cpool.tile([G, 1], DT.float32)
            nc.vector.tensor_reduce(mx[:], z2sb[:], mybir.AxisListType.X, AOT.max)
            negmx = cpool.tile([G, 1], DT.float32)
            nc.vector.tensor_scalar(negmx[:], mx[:], -1.0, None, AOT.mult)
            expd = cpool.tile([G, C], DT.float32)
            sume = cpool.tile([G, 1], DT.float32)
            nc.scalar.activation(expd[:], z2sb[:], ACT.Exp, bias=negmx[:], accum_out=sume[:])
            lse = cpool.tile([G, 1], DT.float32)
            nc.scalar.activation(lse[:], sume[:], ACT.Ln)
            outs = cpool.tile([G, C], DT.float32)
            nc.vector.tensor_scalar(outs[:], z2sb[:], negmx[:], lse[:], AOT.add, AOT.subtract)
            nc.sync.dma_start(out_dram.ap()[:], outs[:])

    nc.compile()
    return nc


def _prep_inputs(x, edge_index, batch, W_mlp, b_mlp, bn_gamma, bn_beta,
                 bn_mean, bn_var, Wc1, bc1, Wc2, bc2):
    """Host-side preprocessing: node permutation, edge grouping, weight folding."""
    row = edge_index[0].astype(np.int64)
    col = edge_index[1].astype(np.int64)
    mask = row != col
    rr, cc = row[mask], col[mask]
    indeg = np.bincount(rr, minlength=N0)
    dv = indeg + 1.0

    # balance per-tile edge load: snake-deal nodes by (indeg+1) desc
    deg_all = np.zeros(NPAD)
    deg_all[:N0] = dv
    order = np.argsort(-deg_all, kind="stable")
    snake = np.concatenate([np.arange(NTILES), np.arange(NTILES)[::-1]])
    tile_seq = np.tile(snake, NPAD // (2 * NTILES))[:NPAD]
    idx_sorted = np.argsort(tile_seq, kind="stable")
    slots = np.empty(NPAD, np.int64)
    slots[idx_sorted] = np.arange(NPAD) - np.repeat(np.arange(NTILES) * 128, 128)
    new_id = np.empty(NPAD, np.int64)
    new_id[order] = tile_seq * 128 + slots
    pi = new_id[:N0]

    # edge lists (non-self + self edges), grouped by (dest tile, src half)
    er = np.concatenate([pi[rr], pi[np.arange(N0)]])
    ec = np.concatenate([pi[cc], pi[np.arange(N0)]])
    half = (ec >= HALF).astype(np.int64)
    grp = (er // 128) * 2 + half
    cnt = np.bincount(grp, minlength=NTILES * 2)
    assert cnt.max() <= CAPH, f"edge group overflow: {cnt.max()} > {CAPH}"
    eorder = np.argsort(grp, kind="stable")
    er_s, ec_s = er[eorder], ec[eorder]
    starts = np.zeros(NTILES * 2 + 1, np.int64)
    starts[1:] = np.cumsum(cnt)

    idx16 = np.zeros((CORES, 128, NT * 2 * SLOTS16), np.int16)
    rowlocv = np.full((CORES, 128, NT * NCH), -1.0, np.float32)
    for c in range(CORES):
        for t in range(NT):
            gt = c * NT + t
            for h in range(2):
                g = gt * 2 + h
                lo, hi = starts[g], starts[g + 1]
                n = hi - lo
                e = np.arange(n)
                base16 = np.zeros((16, SLOTS16), np.int16)
                base16[e % 16, e // 16] = (ec_s[lo:hi] - h * HALF).astype(np.int16)
                idx16[c, :, (t * 2 + h) * SLOTS16:(t * 2 + h + 1) * SLOTS16] = np.tile(base16, (8, 1))
                rowlocv[c, e % 128, t * NCH + 8 * h + e // 128] = (er_s[lo:hi] % 128).astype(np.float32)

    deg_new = np.zeros(NPAD, np.float32)
    deg_new[pi] = dv
    batch_new = np.full(NPAD, -1.0, np.float32)
    batch_new[pi] = batch.astype(np.float32)
    degones = np.stack([np.ones(NPAD, np.float32), deg_new], 0).reshape(2, CORES, NPC).transpose(1, 0, 2)
    batchloc = batch_new.reshape(CORES, NT, 128).transpose(0, 2, 1)

    # fold BN into weights (fp64)
    s_bn = bn_gamma.astype(np.float64) / np.sqrt(bn_var.astype(np.float64) + BN_EPS)
    bb = bn_beta.astype(np.float64) - bn_mean.astype(np.float64) * s_bn
    wmlp = np.zeros((2 * L, F, F), np.float64)
    biaslv = np.zeros((L, 3, F), np.float64)
    for k in range(L):
        sp = np.ones(F) if k == 0 else s_bn[k - 1, 1]
        bp = np.zeros(F) if k == 0 else bb[k - 1, 1]
        W1 = W_mlp[k, 0].astype(np.float64)
        W2 = W_mlp[k, 1].astype(np.float64)
        wmlp[2 * k] = sp[:, None] * W1
        wmlp[2 * k + 1] = s_bn[k, 0][:, None] * W2
        biaslv[k, 0] = b_mlp[k, 0].astype(np.float64)
        biaslv[k, 1] = bp @ W1
        biaslv[k, 2] = b_mlp[k, 1].astype(np.float64) + bb[k, 0] @ W2
    # NOTE: bc1/bc2 are zeros in setup_inputs; folded classifier ignores them
    # except adding bc1/bc2 would need extra matmuls. Assert and fold into zfix
    # is not possible (per-graph). They are zero; verify:
    assert np.abs(bc1).max() == 0.0 and np.abs(bc2).max() == 0.0

    n_g = np.bincount(batch.astype(np.int64), minlength=G).astype(np.float64)
    sfix = np.ones((5, F), np.float64)
    zfixv = np.zeros((5, F, G), np.float64)
    for k in range(L):
        sfix[k + 1] = s_bn[k, 1]
        zfixv[k + 1] = bb[k, 1][:, None] * n_g[None, :]

    x_perm = np.zeros((NPAD, F), np.float32)
    x_perm[pi] = x
    xt = np.ascontiguousarray(x_perm.astype(bf16))

    jrowv = np.tile(np.arange(128, dtype=np.float32)[None, :], (128, 1))
    identv = np.eye(128, dtype=np.float32)

    shared = {
        "xtab": xt,
        "wmlp": wmlp.astype(bf16),
        "biasl": biaslv.astype(bf16),
        "wc1": np.ascontiguousarray(
            Wc1.astype(np.float32).reshape(5, F, 5, F).transpose(0, 2, 1, 3).reshape(25, F, F)
        ),
        "wc2": np.ascontiguousarray(Wc2.astype(np.float32).reshape(5, F, C)),
        "sfix": np.ascontiguousarray(sfix.T.astype(np.float32)),          # [128,5]
        "zfix": np.ascontiguousarray(zfixv.transpose(1, 0, 2).reshape(F, 5 * G).astype(np.float32)),
        "jrow": jrowv.astype(bf16),
        "ident": identv.astype(bf16),
    }
    in_maps = []
    for c in range(CORES):
        m = dict(shared)
        m["xloc"] = np.ascontiguousarray(xt[c * NPC:(c + 1) * NPC])
        m["idx16"] = np.ascontiguousarray(idx16[c])
        m["rowloc"] = np.ascontiguousarray(rowlocv[c])
        m["degones"] = np.ascontiguousarray(degones[c].astype(bf16))
        m["batchloc"] = np.ascontiguousarray(batchloc[c])
        in_maps.append(m)
    return in_maps


TRACE = False
TMPDIR = None
LAST_RESULT = [None]


def kernel(**inputs):
    if "nc" not in _CACHE:
        _CACHE["nc"] = _build_program()
    nc = _CACHE["nc"]
    in_maps = _prep_inputs(**inputs)
    res = run_bass_kernel_spmd(
        nc, in_maps, core_ids=list(range(CORES)), trace=TRACE, tmpdir=TMPDIR
    )
    LAST_RESULT[0] = res
    return np.asarray(res.results[0]["out"], dtype=np.float32)


# revision 11
# speedup vs baseline: 1.0045x; 1.0045x over previous
"""Trainium2 Bass kernel for the GIN message-passing GNN (8 NeuronCores).

Strategy
--------
Nodes are relabeled (host-side permutation) to balance per-tile edge load and
sharded contiguously across 8 cores (6272 nodes/core = 49 tiles of 128).
Edges are assigned to the core/tile of their DESTINATION node; the GIN
self-loop term becomes an explicit self-edge in the gather list.  Each layer:

  1. `dma_gather` pulls the bf16 source-node features (256B rows) from a
     replicated HBM table (two halves, since gather indices are int16).
  2. One-hot "S" matrices (built on-chip with is_equal against a constant
     iota row) turn the segment-sum into PE matmuls accumulating agg^T in
     PSUM: agg^T[f,m] = sum_e G[e,f] * [row_e == m].
  3. The GIN MLP (2x Linear+ReLU+BN-eval) runs as two 128x128 matmuls; BN
     affine params are folded into the next linear's weights on the host.
     The degree-dependent bias (from folding BN through the aggregation) is
     added with a tiny K=2 matmul against a per-tile {degree, ones} matrix.
  4. A PE transpose produces the node-major tile, which is DMAd into the
     local slice of the next gather table; an AllGather collective rebuilds
     the replicated table for the next layer.
  5. Graph readouts accumulate in PSUM via one-hot batch matmuls; a final
     AllReduce + classifier (fp32) + log_softmax runs on every core.

The tables store the pre-BN relu outputs ("r-basis"); readouts are fixed up
after the AllReduce with the folded scale and a host-computed n_g * beta
constant.
"""

import numpy as np
import ml_dtypes
from contextlib import ExitStack  # noqa: F401

from concourse import bass, bacc, tile, mybir
from concourse.bass_utils import run_bass_kernel_spmd

bf16 = ml_dtypes.bfloat16
DT = mybir.dt

# ---- problem constants (hardcoded per contest contract)
N0, E0, F, L, M, G, C = 50000, 600000, 128, 4, 2, 64, 10
BN_EPS = 1e-5
CORES, NPC, NT, P = 8, 6272, 49, 128
NPAD, HALF = 50176, 25088
CAPH = 1024                  # per-(tile,half) edge capacity (pad w/ idx 0)
SLOTS16 = CAPH // 16         # 64 int16 index columns per gather
NCHH = CAPH // 128           # 8 chunks per half
NCH = 2 * NCHH               # 16 chunks per tile
NTILES = NT * CORES

_CACHE = {}


def _build_program():
    nc = bacc.Bacc(
        "TRN2",
        target_bir_lowering=False,
        debug=False,
        enable_asserts=False,
        num_devices=CORES,
    )

    # ---------------- I/O ----------------
    xtab = nc.dram_tensor("xtab", [NPAD, F], DT.bfloat16, kind="ExternalInput")
    xloc = nc.dram_tensor("xloc", [NPC, F], DT.bfloat16, kind="ExternalInput")
    idx16 = nc.dram_tensor("idx16", [128, NT * 2 * SLOTS16], DT.int16, kind="ExternalInput")
    rowloc = nc.dram_tensor("rowloc", [128, NT * NCH], DT.float32, kind="ExternalInput")
    degones = nc.dram_tensor("degones", [2, NPC], DT.bfloat16, kind="ExternalInput")
    batchloc = nc.dram_tensor("batchloc", [128, NT], DT.float32, kind="ExternalInput")
    wmlp = nc.dram_tensor("wmlp", [2 * L, F, F], DT.bfloat16, kind="ExternalInput")
    biasl = nc.dram_tensor("biasl", [L, 3, F], DT.bfloat16, kind="ExternalInput")
    wc1 = nc.dram_tensor("wc1", [25, F, F], DT.float32, kind="ExternalInput")
    wc2 = nc.dram_tensor("wc2", [5, F, C], DT.float32, kind="ExternalInput")
    sfix = nc.dram_tensor("sfix", [128, 5], DT.float32, kind="ExternalInput")
    zfix = nc.dram_tensor("zfix", [128, 5 * G], DT.float32, kind="ExternalInput")
    jrow = nc.dram_tensor("jrow", [128, 128], DT.bfloat16, kind="ExternalInput")
    ident = nc.dram_tensor("ident", [128, 128], DT.bfloat16, kind="ExternalInput")
    out_dram = nc.dram_tensor("out", [G, C], DT.float32, kind="ExternalOutput")

    # internal DRAM for collectives
    ccin = [nc.dram_tensor(f"ccin{k}", [NPC, F], DT.bfloat16) for k in range(L - 1)]
    ccout = [
        nc.dram_tensor(f"ccout{k}", [NPAD, F], DT.bfloat16, addr_space="Shared")
        for k in range(L - 1)
    ]
    zrin = nc.dram_tensor("zrin", [128, 5 * G], DT.float32)
    zrout = nc.dram_tensor("zrout", [128, 5 * G], DT.float32, addr_space="Shared")

    AOT = mybir.AluOpType
    ACT = mybir.ActivationFunctionType
    gsem = nc.alloc_semaphore("gather_dma")

    with tile.TileContext(nc) as tc:
        with (
            tc.tile_pool(name="const", bufs=1) as cpool,
            tc.tile_pool(name="stage", bufs=6) as stpool,
            tc.tile_pool(name="smat", bufs=3) as spool,
            tc.tile_pool(name="work", bufs=4) as wpool,
            tc.tile_pool(name="psum", bufs=1, space="PSUM") as pspool,
            tc.tile_pool(name="psumr", bufs=1, space="PSUM") as prpool,
        ):
            # ------- resident constants -------
            idx_sb = cpool.tile([128, NT * 2 * SLOTS16], DT.int16)
            nc.sync.dma_start(idx_sb[:], idx16.ap())
            rowloc_sb = cpool.tile([128, NT * NCH], DT.float32)
            nc.sync.dma_start(rowloc_sb[:], rowloc.ap())
            degones_sb = cpool.tile([2, NPC], DT.bfloat16)
            nc.sync.dma_start(degones_sb[:], degones.ap())
            batchloc_sb = cpool.tile([128, NT], DT.float32)
            nc.sync.dma_start(batchloc_sb[:], batchloc.ap())
            jrow_sb = cpool.tile([128, 128], DT.bfloat16)
            nc.sync.dma_start(jrow_sb[:], jrow.ap())
            ident_sb = cpool.tile([128, 128], DT.bfloat16)
            nc.sync.dma_start(ident_sb[:], ident.ap())
            wmlp_sb = cpool.tile([128, 2 * L * F], DT.bfloat16)
            for ki in range(2 * L):
                nc.sync.dma_start(wmlp_sb[:, ki * F:(ki + 1) * F], wmlp.ap()[ki])
            biasl12_sb = cpool.tile([2, L * F], DT.bfloat16)
            biasl3_sb = cpool.tile([1, L * F], DT.bfloat16)
            for k in range(L):
                nc.sync.dma_start(biasl12_sb[:, k * F:(k + 1) * F], biasl.ap()[k][0:2, :])
                nc.sync.dma_start(biasl3_sb[:, k * F:(k + 1) * F], biasl.ap()[k][2:3, :])
            wc1_sb = cpool.tile([128, 25 * F], DT.float32)
            for ij in range(25):
                nc.sync.dma_start(wc1_sb[:, ij * F:(ij + 1) * F], wc1.ap()[ij])
            wc2_sb = cpool.tile([128, 5 * C], DT.float32)
            for j in range(5):
                nc.sync.dma_start(wc2_sb[:, j * C:(j + 1) * C], wc2.ap()[j])
            sfix_sb = cpool.tile([128, 5], DT.float32)
            nc.sync.dma_start(sfix_sb[:], sfix.ap())
            zfix_sb = cpool.tile([128, 5 * G], DT.float32)
            nc.sync.dma_start(zfix_sb[:], zfix.ap())

            # one-hot batch matrices per tile: B[m, g] = (batchloc[m,t]==g)
            ball_sb = cpool.tile([128, NT * G], DT.bfloat16)
            for t in range(NT):
                nc.vector.tensor_scalar(
                    ball_sb[:, t * G:(t + 1) * G],
                    jrow_sb[:, :G],
                    batchloc_sb[:, t:t + 1],
                    None,
                    AOT.is_equal,
                )

            zr_sb = cpool.tile([128, 5 * G], DT.float32)

            # ---------------- GIN layers ----------------
            for k in range(L):
                table = xtab if k == 0 else ccout[k - 1]
                tap = table.ap()
                psR = prpool.tile([128, G], DT.float32, tag="psR")
                psR0 = prpool.tile([128, G], DT.float32, tag="psR0", name="psR0") if k == 0 else None
                for t in range(NT):
                    stg = [
                        stpool.tile([128, NCHH, 128], DT.bfloat16, tag=f"stg{h}",
                                    name=f"stg{h}")
                        for h in range(2)
                    ]
                    for h in range(2):
                        nc.gpsimd.dma_gather(
                            out_ap=stg[h][:],
                            in_ap=tap[h * HALF:(h + 1) * HALF, :],
                            idxs_ap=idx_sb[:, (t * 2 + h) * SLOTS16:(t * 2 + h + 1) * SLOTS16],
                            num_idxs=CAPH,
                            num_idxs_reg=CAPH,
                            elem_size=F,
                            queue_num=0,
                        )
                    smat = spool.tile([128, NCH, 128], DT.bfloat16, tag="smat")
                    for ch in range(NCH):
                        nc.vector.tensor_scalar(
                            smat[:, ch, :],
                            jrow_sb[:],
                            rowloc_sb[:, t * NCH + ch:t * NCH + ch + 1],
                            None,
                            AOT.is_equal,
                        )
                    psA = pspool.tile([128, 128], DT.float32, tag="psA", bufs=2)
                    for ch in range(NCH):
                        nc.tensor.matmul(
                            psA[:],
                            stg[ch // NCHH][:, ch % NCHH, :],
                            smat[:, ch, :],
                            start=(ch == 0),
                            stop=(ch == NCH - 1),
                        )
                    aggr = wpool.tile([128, 128], DT.bfloat16, tag="aggr")
                    nc.scalar.copy(aggr[:], psA[:])
                    # MLP sublayer 1 (+ degree bias)
                    psB = pspool.tile([128, 128], DT.float32, tag="psB")
                    nc.tensor.matmul(
                        psB[:], wmlp_sb[:, (2 * k) * F:(2 * k + 1) * F], aggr[:],
                        start=True, stop=False,
                    )
                    nc.tensor.matmul(
                        psB[:], biasl12_sb[:, k * F:(k + 1) * F],
                        degones_sb[0:2, t * 128:(t + 1) * 128],
                        start=False, stop=True,
                    )
                    r1 = wpool.tile([128, 128], DT.bfloat16, tag="r1")
                    nc.scalar.activation(r1[:], psB[:], ACT.Relu)
                    # MLP sublayer 2 (+ constant bias)
                    psC = pspool.tile([128, 128], DT.float32, tag="psC")
                    nc.tensor.matmul(
                        psC[:], wmlp_sb[:, (2 * k + 1) * F:(2 * k + 2) * F], r1[:],
                        start=True, stop=False,
                    )
                    nc.tensor.matmul(
                        psC[:], biasl3_sb[:, k * F:(k + 1) * F],
                        degones_sb[0:1, t * 128:(t + 1) * 128],
                        start=False, stop=True,
                    )
                    r2 = wpool.tile([128, 128], DT.bfloat16, tag="r2")
                    nc.scalar.activation(r2[:], psC[:], ACT.Relu)
                    # node-major copy (for table store + readout)
                    psT = pspool.tile([128, 128], DT.bfloat16, tag="psT", bufs=2)
                    nc.tensor.matmul(psT[:], r2[:], ident_sb[:], is_transpose=True)
                    r2nm = wpool.tile([128, 128], DT.bfloat16, tag="r2nm")
                    nc.scalar.copy(r2nm[:], psT[:])
                    if k < L - 1:
                        nc.sync.dma_start(ccin[k].ap()[t * 128:(t + 1) * 128, :], r2nm[:])
                    nc.tensor.matmul(
                        psR[:], r2nm[:], ball_sb[:, t * G:(t + 1) * G],
                        start=(t == 0), stop=(t == NT - 1), skip_group_check=True,
                    )
                    if k == 0:
                        xt_t = stpool.tile([128, 128], DT.bfloat16, tag="xt")
                        nc.sync.dma_start(xt_t[:], xloc.ap()[t * 128:(t + 1) * 128, :])
                        nc.tensor.matmul(
                            psR0[:], xt_t[:], ball_sb[:, t * G:(t + 1) * G],
                            start=(t == 0), stop=(t == NT - 1), skip_group_check=True,
                        )
                nc.scalar.copy(zr_sb[:, (k + 1) * G:(k + 2) * G], psR[:])
                if k == 0:
                    nc.scalar.copy(zr_sb[:, 0:G], psR0[:])
                if k < L - 1:
                    nc.gpsimd.collective_compute(
                        "AllGather",
                        AOT.bypass,
                        replica_groups=[list(range(CORES))],
                        ins=[ccin[k].ap().opt()],
                        outs=[ccout[k].ap().opt()],
                    )

            # ---------------- readout AllReduce + fixup ----------------
            nc.sync.dma_start(zrin.ap()[:], zr_sb[:])
            nc.gpsimd.collective_compute(
                "AllReduce",
                AOT.add,
                replica_groups=[list(range(CORES))],
                ins=[zrin.ap().opt()],
                outs=[zrout.ap().opt()],
            )
            zsum = cpool.tile([128, 5 * G], DT.float32)
            nc.sync.dma_start(zsum[:], zrout.ap()[:])
            zfx = cpool.tile([128, 5 * G], DT.float32)
            for kk in range(5):
                nc.vector.tensor_scalar(
                    zfx[:, kk * G:(kk + 1) * G],
                    zsum[:, kk * G:(kk + 1) * G],
                    sfix_sb[:, kk:kk + 1],
                    None,
                    AOT.mult,
                )
            nc.vector.tensor_tensor(zfx[:], zfx[:], zfix_sb[:], AOT.add)

            # ---------------- classifier (fp32) ----------------
            rc1 = []
            for j in range(5):
                psC1 = pspool.tile([128, G], DT.float32, tag="psA", name="psC1", bufs=2)
                for i in range(5):
                    nc.tensor.matmul(
                        psC1[:], wc1_sb[:, (i * 5 + j) * F:(i * 5 + j + 1) * F],
                        zfx[:, i * G:(i + 1) * G],
                        start=(i == 0), stop=(i == 4),
                    )
                r = cpool.tile([128, G], DT.float32, tag=f"rc1_{j}", name=f"rc1_{j}")
                nc.scalar.activation(r[:], psC1[:], ACT.Relu)
                rc1.append(r)
            psC2 = prpool.tile([G, C], DT.float32, tag="psR0", name="psC2")
            for j in range(5):
                nc.tensor.matmul(
                    psC2[:], rc1[j][:], wc2_sb[:, j * C:(j + 1) * C],
                    start=(j == 0), stop=(j == 4),
                )
            z2sb = cpool.tile([G, C], DT.float32)
            nc.scalar.copy(z2sb[:], psC2[:])
            mx = cpool.tile([G, 1], DT.float32)
            nc.vector.tensor_reduce(mx[:], z2sb[:], mybir.AxisListType.X, AOT.max)
            negmx = cpool.tile([G, 1], DT.float32)
            nc.vector.tensor_scalar(negmx[:], mx[:], -1.0, None, AOT.mult)
            expd = cpool.tile([G, C], DT.float32)
            sume = cpool.tile([G, 1], DT.float32)
            nc.scalar.activation(expd[:], z2sb[:], ACT.Exp, bias=negmx[:], accum_out=sume[:])
            lse = cpool.tile([G, 1], DT.float32)
            nc.scalar.activation(lse[:], sume[:], ACT.Ln)
            outs = cpool.tile([G, C], DT.float32)
            nc.vector.tensor_scalar(outs[:], z2sb[:], negmx[:], lse[:], AOT.add, AOT.subtract)
            nc.sync.dma_start(out_dram.ap()[:], outs[:])

    nc.compile()
    return nc


def _prep_inputs(x, edge_index, batch, W_mlp, b_mlp, bn_gamma, bn_beta,
                 bn_mean, bn_var, Wc1, bc1, Wc2, bc2):
    """Host-side preprocessing: node permutation, edge grouping, weight folding."""
    row = edge_index[0].astype(np.int64)
    col = edge_index[1].astype(np.int64)
    mask = row != col
    rr, cc = row[mask], col[mask]
    indeg = np.bincount(rr, minlength=N0)
    dv = indeg + 1.0

    # balance per-tile edge load: snake-deal nodes by (indeg+1) desc
    deg_all = np.zeros(NPAD)
    deg_all[:N0] = dv
    order = np.argsort(-deg_all, kind="stable")
    snake = np.concatenate([np.arange(NTILES), np.arange(NTILES)[::-1]])
    tile_seq = np.tile(snake, NPAD // (2 * NTILES))[:NPAD]
    idx_sorted = np.argsort(tile_seq, kind="stable")
    slots = np.empty(NPAD, np.int64)
    slots[idx_sorted] = np.arange(NPAD) - np.repeat(np.arange(NTILES) * 128, 128)
    new_id = np.empty(NPAD, np.int64)
    new_id[order] = tile_seq * 128 + slots
    pi = new_id[:N0]

    # edge lists (non-self + self edges), grouped by (dest tile, src half)
    er = np.concatenate([pi[rr], pi[np.arange(N0)]])
    ec = np.concatenate([pi[cc], pi[np.arange(N0)]])
    half = (ec >= HALF).astype(np.int64)
    grp = (er // 128) * 2 + half
    cnt = np.bincount(grp, minlength=NTILES * 2)
    assert cnt.max() <= CAPH, f"edge group overflow: {cnt.max()} > {CAPH}"
    eorder = np.argsort(grp, kind="stable")
    er_s, ec_s = er[eorder], ec[eorder]
    starts = np.zeros(NTILES * 2 + 1, np.int64)
    starts[1:] = np.cumsum(cnt)

    idx16 = np.zeros((CORES, 128, NT * 2 * SLOTS16), np.int16)
    rowlocv = np.full((CORES, 128, NT * NCH), -1.0, np.float32)
    for c in range(CORES):
        for t in range(NT):
            gt = c * NT + t
            for h in range(2):
                g = gt * 2 + h
                lo, hi = starts[g], starts[g + 1]
                n = hi - lo
                e = np.arange(n)
                base16 = np.zeros((16, SLOTS16), np.int16)
                base16[e % 16, e // 16] = (ec_s[lo:hi] - h * HALF).astype(np.int16)
                idx16[c, :, (t * 2 + h) * SLOTS16:(t * 2 + h + 1) * SLOTS16] = np.tile(base16, (8, 1))
                rowlocv[c, e % 128, t * NCH + 8 * h + e // 128] = (er_s[lo:hi] % 128).astype(np.float32)

    deg_new = np.zeros(NPAD, np.float32)
    deg_new[pi] = dv
    batch_new = np.full(NPAD, -1.0, np.float32)
    batch_new[pi] = batch.astype(np.float32)
    degones = np.stack([np.ones(NPAD, np.float32), deg_new], 0).reshape(2, CORES, NPC).transpose(1, 0, 2)
    batchloc = batch_new.reshape(CORES, NT, 128).transpose(0, 2, 1)

    # fold BN into weights (fp64)
    s_bn = bn_gamma.astype(np.float64) / np.sqrt(bn_var.astype(np.float64) + BN_EPS)
    bb = bn_beta.astype(np.float64) - bn_mean.astype(np.float64) * s_bn
    wmlp = np.zeros((2 * L, F, F), np.float64)
    biaslv = np.zeros((L, 3, F), np.float64)
    for k in range(L):
        sp = np.ones(F) if k == 0 else s_bn[k - 1, 1]
        bp = np.zeros(F) if k == 0 else bb[k - 1, 1]
        W1 = W_mlp[k, 0].astype(np.float64)
        W2 = W_mlp[k, 1].astype(np.float64)
        wmlp[2 * k] = sp[:, None] * W1
        wmlp[2 * k + 1] = s_bn[k, 0][:, None] * W2
        biaslv[k, 0] = b_mlp[k, 0].astype(np.float64)
        biaslv[k, 1] = bp @ W1
        biaslv[k, 2] = b_mlp[k, 1].astype(np.float64) + bb[k, 0] @ W2
    # NOTE: bc1/bc2 are zeros in setup_inputs; folded classifier ignores them
    # except adding bc1/bc2 would need extra matmuls. Assert and fold into zfix
    # is not possible (per-graph). They are zero; verify:
    assert np.abs(bc1).max() == 0.0 and np.abs(bc2).max() == 0.0

    n_g = np.bincount(batch.astype(np.int64), minlength=G).astype(np.float64)
    sfix = np.ones((5, F), np.float64)
    zfixv = np.zeros((5, F, G), np.float64)
    for k in range(L):
        sfix[k + 1] = s_bn[k, 1]
        zfixv[k + 1] = bb[k, 1][:, None] * n_g[None, :]

    x_perm = np.zeros((NPAD, F), np.float32)
    x_perm[pi] = x
    xt = np.ascontiguousarray(x_perm.astype(bf16))

    jrowv = np.tile(np.arange(128, dtype=np.float32)[None, :], (128, 1))
    identv = np.eye(128, dtype=np.float32)

    shared = {
        "xtab": xt,
        "wmlp": wmlp.astype(bf16),
        "biasl": biaslv.astype(bf16),
        "wc1": np.ascontiguousarray(
            Wc1.astype(np.float32).reshape(5, F, 5, F).transpose(0, 2, 1, 3).reshape(25, F, F)
        ),
        "wc2": np.ascontiguousarray(Wc2.astype(np.float32).reshape(5, F, C)),
        "sfix": np.ascontiguousarray(sfix.T.astype(np.float32)),          # [128,5]
        "zfix": np.ascontiguousarray(zfixv.transpose(1, 0, 2).reshape(F, 5 * G).astype(np.float32)),
        "jrow": jrowv.astype(bf16),
        "ident": identv.astype(bf16),
    }
    in_maps = []
    for c in range(CORES):
        m = dict(shared)
        m["xloc"] = np.ascontiguousarray(xt[c * NPC:(c + 1) * NPC])
        m["idx16"] = np.ascontiguousarray(idx16[c])
        m["rowloc"] = np.ascontiguousarray(rowlocv[c])
        m["degones"] = np.ascontiguousarray(degones[c].astype(bf16))
        m["batchloc"] = np.ascontiguousarray(batchloc[c])
        in_maps.append(m)
    return in_maps


TRACE = False
TMPDIR = None
LAST_RESULT = [None]


def kernel(**inputs):
    if "nc" not in _CACHE:
        _CACHE["nc"] = _build_program()
    nc = _CACHE["nc"]
    in_maps = _prep_inputs(**inputs)
    res = run_bass_kernel_spmd(
        nc, in_maps, core_ids=list(range(CORES)), trace=TRACE, tmpdir=TMPDIR
    )
    LAST_RESULT[0] = res
    return np.asarray(res.results[0]["out"], dtype=np.float32)
